# revision 7
# baseline (speedup 1.0000x reference)
"""GCNConvNet Trainium2 kernel (8 NeuronCores, Bass/Tile).

Dst-sharded graph parallelism, 8 aggregation rounds (A(HW) == (AH)W lets every
round aggregate 64-feature rows):
  - Node features live in an HBM table of bf16 rows padded to 256B (gather
    granule).  Each core owns 12800 destination rows.  The table is split into
    4 quarter tensors (per-core quarter slices interleaved) so the per-round
    AllGather is quartered and overlaps the next round's gathers.
  - Per round, each core gathers its edges' source rows with dma_gather
    (int16 indices; 4 SWDGE queues round-robined), then segment-sums them
    into 64-dst PSUM windows with TensorE matmuls against one-hot scatter
    blocks built ON-CHIP by the vector engine from compact (offset, norm)
    tables (symmetric-norm coefficients folded into the one-hot values).
  - Dense layer transform + bias/activation runs on each 512-dst chunk right
    after its aggregation; the updated chunk is transposed (TensorE) and
    published into the quarter slice; a quarter's AllGather fires as soon as
    its groups are published.
The block schedule is shared by all cores (single NEFF); per-core differences
live entirely in the input tensors (indices, off/norm tables, x shard).
"""

import sys

sys.path.insert(0, "/opt/trn_rl_repo")

import numpy as np
import ml_dtypes

import concourse.bacc as bacc
import concourse.mybir as mybir
import concourse.tile as tile
from concourse.bass_utils import run_bass_kernel_spmd
from concourse.masks import make_identity

P = 128
HID = 64
FW = 128          # table row width (bf16) = 256B gather granule; cols 64: pad
AFT = mybir.ActivationFunctionType
ALU = mybir.AluOpType

N = 100000
NCORES = 8
DPC = 12800       # dst rows per core
W = 64            # dsts per window
NW = 200          # windows per core
WPG = 8           # windows per psum group (512 cols)
NG = 25           # psum groups per core
NROUNDS = 8
J = 100           # 128-row tiles per core slice
QB = np.array([0, 3072, 6144, 9216, 12288, 12800])  # slice row bounds (local)
QS = np.diff(QB)                               # [3072,3072,3072,3072,512]
BK = 8 * QS                                    # bank rows (table slice sizes)
G2Q = [0] * 6 + [1] * 6 + [2] * 6 + [3] * 6 + [4]   # psum group -> slice
# collective for slice q fires after group CGI[q] (q4 gates the next round,
# so it is kept tiny: one group = 1MB AllGather)
CGI = {8: 0, 14: 1, 20: 2, 24: 3}
NB = 5


# ---------------------------------------------------------------- host side --
def preprocess(edge_index):
    """Slot/scatter schedule shared by all cores + per-core idx / off / norm.

    Slot order: (psum-group g, bank b, window w, dst, edge).  Within each
    (g,b): per-window slot counts are equalized across cores (max), then the
    (g,b) range is padded to a multiple of 128.  Slot s maps to m-tile
    position (lane s%128, col s//128).

    Table layout: node n -> k=n//DPC, r=n%DPC, quarter q of r, j=r-QB[q];
    table row (within bank q) = k*QS[q] + j.  Bank b == quarter b.
    """
    src = np.concatenate([edge_index[0], np.arange(N)]).astype(np.int64)
    dst = np.concatenate([edge_index[1], np.arange(N)]).astype(np.int64)
    deg = np.bincount(dst, minlength=N).astype(np.float64)
    dinv = deg ** -0.5
    norm = (dinv[src] * dinv[dst]).astype(np.float32)

    core = dst // DPC
    r = dst % DPC
    win = r // W                    # global window id [0, NW)
    grp = win // WPG                # psum group [0, NG)
    dloc = r % W

    ks = src // DPC
    rs = src % DPC
    qsrc = np.searchsorted(QB, rs, side="right") - 1
    bank = qsrc
    rowin = ks * QS[qsrc] + (rs - QB[qsrc])    # row within bank

    # per-(core, g, b, w) counts -> equalized across cores
    key = ((core * NG + grp) * NB + bank) * NW + win
    cnt = np.bincount(key, minlength=NCORES * NG * NB * NW).reshape(
        NCORES, NG, NB, NW
    )
    cnt_eq = cnt.max(axis=0)                   # [NG, NB, NW]

    gb_sz = cnt_eq.sum(axis=2).reshape(-1)     # [NG*NB], ordered (g, b)
    gb_pad = (-gb_sz) % P
    gb_base = np.concatenate([[0], np.cumsum(gb_sz + gb_pad)])
    NSLOT = int(gb_base[-1])
    NCOL = NSLOT // P

    w_off = np.zeros_like(cnt_eq)
    w_off[:, :, 1:] = np.cumsum(cnt_eq, axis=2)[:, :, :-1]
    w_base = gb_base[:-1].reshape(NG, NB)[:, :, None] + w_off  # [NG,NB,NW]

    # edge -> slot
    order = np.lexsort((dst, win, bank, grp, core))
    srcs_o = src[order]
    rowin_o = rowin[order]
    norms_o = norm[order]
    cores_o, grps_o, banks_o, wins_o, dlocs_o = (
        core[order], grp[order], bank[order], win[order], dloc[order]
    )
    key_o = ((cores_o * NG + grps_o) * NB + banks_o) * NW + wins_o
    starts = np.zeros(NCORES * NG * NB * NW + 1, np.int64)
    np.add.at(starts, key_o + 1, 1)
    starts = np.cumsum(starts)
    rank = np.arange(len(key_o)) - starts[key_o]
    slot = w_base[grps_o, banks_o, wins_o] + rank

    lane = slot % P
    col = slot // P

    idx = np.zeros((NCORES, P, NCOL), np.int16)
    idx[cores_o, lane, col] = rowin_o.astype(np.int16)

    # ---- matmul schedule (shared) ----
    # entry per (g, b, w, col-touched); rhs is always S[:, m, 0:64].
    mm_col, mm_w, mm_g, mm_b = [], [], [], []
    for g in range(NG):
        for b in range(NB):
            for w in range(WPG * g, WPG * (g + 1)):
                base = int(w_base[g, b, w])
                n = int(cnt_eq[g, b, w])
                if n == 0:
                    continue
                for c in range(base // P, (base + n - 1) // P + 1):
                    mm_col.append(c)
                    mm_w.append(w)
                    mm_g.append(g)
                    mm_b.append(b)
    NMM = len(mm_col)
    first_of_w, last_of_w = {}, {}
    for m, w in enumerate(mm_w):
        if w not in first_of_w:
            first_of_w[w] = m
        last_of_w[w] = m
    mm_start = np.array([first_of_w[w] == m for m, w in enumerate(mm_w)])
    mm_stop = np.array([last_of_w[w] == m for m, w in enumerate(mm_w)])

    # compact S-build tables: off (dst offset in window) + norm per (lane, m)
    mm_of = {}
    for m in range(NMM):
        mm_of[(mm_col[m], mm_w[m])] = m
    pair_keys = col * NW + wins_o
    uniq, inv = np.unique(pair_keys, return_inverse=True)
    mm_u = np.array([mm_of[(int(pk) // NW, int(pk) % NW)] for pk in uniq])
    m_of_edge = mm_u[inv]
    off_arr = np.zeros((NCORES, P, NMM), np.float32)
    nrm_arr = np.zeros((NCORES, P, NMM), np.float32)
    off_arr[cores_o, lane, m_of_edge] = dlocs_o
    nrm_arr[cores_o, lane, m_of_edge] = norms_o

    groups = []
    for g in range(NG):
        gc0 = int(gb_base[g * NB] // P)
        gc1 = int(gb_base[(g + 1) * NB] // P) if g + 1 < NG else NCOL
        bank_cols = [
            (int(gb_base[g * NB + b] // P), int(gb_base[g * NB + b + 1] // P))
            for b in range(NB)
        ]
        mms = [m for m in range(NMM) if mm_g[m] == g]
        groups.append(
            dict(cols=(gc0, gc1), bank_cols=bank_cols,
                 mm0=min(mms), mm1=max(mms) + 1)
        )
    for g, gr in enumerate(groups):
        for m in range(gr["mm0"], gr["mm1"]):
            assert mm_g[m] == g

    sched = dict(
        mm_col=mm_col, mm_w=mm_w, mm_b=mm_b,
        mm_start=mm_start, mm_stop=mm_stop,
        groups=groups, NMM=NMM, NCOL=NCOL, NSLOT=NSLOT,
    )
    return dict(
        idx=idx,
        off=off_arr.astype(ml_dtypes.bfloat16),
        nrm=nrm_arr.astype(ml_dtypes.bfloat16),
        sched=sched,
    )


# -------------------------------------------------------------- device side --
def build_program(sched):
    f32, bf16, i16 = mybir.dt.float32, mybir.dt.bfloat16, mybir.dt.int16
    mm_col, mm_w, mm_b = sched["mm_col"], sched["mm_w"], sched["mm_b"]
    mm_start, mm_stop = sched["mm_start"], sched["mm_stop"]
    groups, NMM, NCOL = sched["groups"], sched["NMM"], sched["NCOL"]
    NHID = NROUNDS - 2

    nc = bacc.Bacc(
        "TRN2", target_bir_lowering=False, debug=False,
        num_devices=NCORES, num_swdge_queues=4,
    )

    idx_t = nc.dram_tensor("idx", [P, NCOL * 8], i16, kind="ExternalInput")
    off_t = nc.dram_tensor("offt", [P, NMM], bf16, kind="ExternalInput")
    nrm_t = nc.dram_tensor("nrmt", [P, NMM], bf16, kind="ExternalInput")
    x_t = nc.dram_tensor("xsh", [DPC, 3], f32, kind="ExternalInput")
    win_t = nc.dram_tensor("w_in", [3, HID], f32, kind="ExternalInput")
    bin_t = nc.dram_tensor("b_in", [HID, 1], f32, kind="ExternalInput")
    whid_t = nc.dram_tensor("w_hid", [NHID, HID, HID], bf16, kind="ExternalInput")
    bhid_t = nc.dram_tensor("b_hid", [NHID, HID, 1], f32, kind="ExternalInput")
    wout_t = nc.dram_tensor("w_out", [HID, 6], bf16, kind="ExternalInput")
    bout_t = nc.dram_tensor("b_out", [6, 1], f32, kind="ExternalInput")
    y_t = nc.dram_tensor("y", [DPC, 6], f32, kind="ExternalOutput")

    # sliced ping-pong tables + publish slices
    tq = [
        [
            nc.dram_tensor(f"table{i}q{q}", [int(BK[q]), FW], bf16,
                           addr_space="Shared")
            for q in range(NB)
        ]
        for i in range(2)
    ]
    hq = [
        [nc.dram_tensor(f"hsl{i}q{q}", [int(QS[q]), FW], bf16) for q in range(NB)]
        for i in range(2)
    ]
    rg = [list(range(NCORES))]

    GC = max(gr["cols"][1] - gr["cols"][0] for gr in groups)
    SM = max(gr["mm1"] - gr["mm0"] for gr in groups)
    for gr in groups:
        for b in range(NB):
            c0, c1 = gr["bank_cols"][b]
            assert (c1 - c0) * P <= 8192

    with tile.TileContext(nc, num_cores=NCORES) as tc:
        with (
            tc.tile_pool(name="const", bufs=1) as cpool,
            tc.tile_pool(name="mp", bufs=2) as mpool,
            tc.tile_pool(name="sp", bufs=2) as spool,
            tc.tile_pool(name="ip", bufs=3) as ipool,
            tc.tile_pool(name="trp", bufs=2) as trpool,
            tc.tile_pool(name="rhp", bufs=2) as rhpool,
            tc.tile_pool(name="atp", bufs=2) as atpool,
            tc.tile_pool(name="ps_sc", bufs=2, space="PSUM") as ps_sc,
            tc.tile_pool(name="ps_tr", bufs=2, space="PSUM") as ps_tr,
            tc.tile_pool(name="ps_tp", bufs=2, space="PSUM") as ps_tp,
        ):
            # ---- constants ----
            ident_f = cpool.tile([P, P], f32, tag="idf")
            make_identity(nc, ident_f[:])
            ident_b = cpool.tile([P, P], bf16, tag="idb")
            make_identity(nc, ident_b[:])
            w_in_sb = cpool.tile([3, HID], f32, tag="wi")
            nc.sync.dma_start(out=w_in_sb[:], in_=win_t[:])
            b_in_sb = cpool.tile([HID, 1], f32, tag="bi")
            nc.sync.dma_start(out=b_in_sb[:], in_=bin_t[:])
            whid_sb = cpool.tile([HID, NHID * HID], bf16, tag="wh")
            bhid_sb = cpool.tile([HID, NHID], f32, tag="bh")
            for l in range(NHID):
                nc.sync.dma_start(
                    out=whid_sb[:, l * HID : (l + 1) * HID], in_=whid_t[l, :, :]
                )
                nc.sync.dma_start(out=bhid_sb[:, l : l + 1], in_=bhid_t[l, :, :])
            wout_sb = cpool.tile([HID, 6], bf16, tag="wo")
            nc.sync.dma_start(out=wout_sb[:], in_=wout_t[:])
            bout_sb = cpool.tile([6, 1], f32, tag="bo")
            nc.sync.dma_start(out=bout_sb[:], in_=bout_t[:])
            iota_sb = cpool.tile([P, W], bf16, tag="io")
            nc.gpsimd.iota(
                iota_sb[:], pattern=[[1, W]], base=0, channel_multiplier=0,
                allow_small_or_imprecise_dtypes=True,
            )
            off_sb = cpool.tile([P, NMM], bf16, tag="of")
            nc.sync.dma_start(out=off_sb[:], in_=off_t[:])
            nrm_sb = cpool.tile([P, NMM], bf16, tag="nr")
            nc.sync.dma_start(out=nrm_sb[:], in_=nrm_t[:])

            # ---- round 0 table: t0 = x @ W_in, published per slice ----
            for q in range(NB):
                jt0, nj = int(QB[q] // P), int(QS[q] // P)
                t0 = trpool.tile([P, 24 * FW], bf16, tag="t0")
                nc.vector.memset(t0[:, 0 : nj * FW], 0.0)
                for jj in range(nj):
                    j = jt0 + jj
                    xc = rhpool.tile([P, 3], f32, tag="xc")
                    nc.sync.dma_start(out=xc[:], in_=x_t[j * P : (j + 1) * P, :])
                    pxT = ps_tp.tile([3, P], f32, space="PSUM", tag="ptp")
                    nc.tensor.transpose(out=pxT[:], in_=xc[:], identity=ident_f[:])
                    xT = rhpool.tile([3, P], f32, tag="xT")
                    nc.vector.tensor_copy(out=xT[:], in_=pxT[:])
                    pt0 = ps_tr.tile([P, HID], f32, space="PSUM", tag="ptr")
                    nc.tensor.matmul(
                        out=pt0[:], lhsT=xT[:], rhs=w_in_sb[:], start=True,
                        stop=True,
                    )
                    nc.scalar.activation(
                        out=t0[:, jj * FW : jj * FW + HID], in_=pt0[:],
                        func=AFT.Copy,
                    )
                nc.sync.dma_start(
                    out=hq[0][q].ap().rearrange("(j p) f -> p j f", p=P),
                    in_=t0[:, 0 : nj * FW].rearrange("p (j f) -> p j f", f=FW),
                )
                nc.gpsimd.collective_compute(
                    "AllGather", mybir.AluOpType.bypass, replica_groups=rg,
                    ins=[hq[0][q][:]], outs=[tq[0][q][:, :]],
                )

            # ---- rounds ----
            qn = 0
            for r in range(NROUNDS):
                ti, tn = r % 2, (r + 1) % 2
                for gi, gr in enumerate(groups):
                    gc0, gc1 = gr["cols"]
                    # gathers (4 banks -> one m tile)
                    idx_sb = ipool.tile([P, GC * 8], i16, tag="ix")
                    nc.sync.dma_start(
                        out=idx_sb[:, 0 : (gc1 - gc0) * 8],
                        in_=idx_t[:, gc0 * 8 : gc1 * 8],
                    )
                    mt = mpool.tile([P, GC, FW], bf16, tag="m")
                    for b in range(NB):
                        c0, c1 = gr["bank_cols"][b]
                        if c1 == c0:
                            continue
                        nidx = (c1 - c0) * P
                        nc.gpsimd.dma_gather(
                            out_ap=mt[:, c0 - gc0 : c1 - gc0, :],
                            in_ap=tq[ti][b][:, :],
                            idxs_ap=idx_sb[:, (c0 - gc0) * 8 : (c1 - gc0) * 8],
                            num_idxs=nidx,
                            num_idxs_reg=nidx,
                            elem_size=FW,
                            single_packet=False,
                            queue_num=qn % 4,
                        )
                        qn += 1
                    # on-chip S build: S = (iota == off) * norm
                    mm0, mm1 = gr["mm0"], gr["mm1"]
                    nm = mm1 - mm0
                    sg = spool.tile([P, SM, W], bf16, tag="s")
                    iota_b = (
                        iota_sb[:]
                        .rearrange("p (o j) -> p o j", o=1)
                        .broadcast_to([P, nm, W])
                    )
                    off_b = (
                        off_sb[:, mm0:mm1]
                        .rearrange("p (m o) -> p m o", o=1)
                        .broadcast_to([P, nm, W])
                    )
                    nrm_b = (
                        nrm_sb[:, mm0:mm1]
                        .rearrange("p (m o) -> p m o", o=1)
                        .broadcast_to([P, nm, W])
                    )
                    nc.vector.scalar_tensor_tensor(
                        out=sg[:, 0:nm, :], in0=iota_b, scalar=0.0, in1=off_b,
                        op0=ALU.add, op1=ALU.is_equal,
                    )
                    nc.vector.scalar_tensor_tensor(
                        out=sg[:, 0:nm, :], in0=sg[:, 0:nm, :], scalar=1.0,
                        in1=nrm_b, op0=ALU.mult, op1=ALU.mult,
                    )
                    # aggregation matmuls
                    psum = ps_sc.tile([HID, 512], f32, space="PSUM", tag="psc")
                    for m in range(mm0, mm1):
                        c, w = mm_col[m], mm_w[m]
                        wl = w - WPG * gi
                        nc.tensor.matmul(
                            out=psum[:, wl * W : (wl + 1) * W],
                            lhsT=mt[:, c - gc0, 0:HID],
                            rhs=sg[:, m - mm0, 0:W],
                            start=bool(mm_start[m]),
                            stop=bool(mm_stop[m]),
                            skip_group_check=True,
                        )
                    # transform + transpose + publish for this 512-dst chunk
                    if r == NROUNDS - 1:
                        atc = atpool.tile([HID, 512], bf16, tag="at")
                        nc.scalar.activation(
                            out=atc[:], in_=psum[:], func=AFT.Copy,
                        )
                        yc = rhpool.tile([6, 512], f32, tag="yc")
                        pt6 = ps_tr.tile([6, 512], f32, space="PSUM", tag="ptr")
                        nc.tensor.matmul(
                            out=pt6[:], lhsT=wout_sb[:], rhs=atc[:],
                            start=True, stop=True,
                        )
                        nc.scalar.activation(
                            out=yc[:], in_=pt6[:], func=AFT.Sigmoid,
                            bias=bout_sb[:],
                        )
                        ytg = trpool.tile([P, 4 * 6], f32, tag="yt")
                        for jj in range(4):
                            ptp6 = ps_tp.tile([P, 6], f32, space="PSUM", tag="ptp")
                            nc.tensor.transpose(
                                out=ptp6[:], in_=yc[:, jj * P : (jj + 1) * P],
                                identity=ident_f[0:6, 0:6],
                            )
                            nc.vector.tensor_copy(
                                out=ytg[:, jj * 6 : (jj + 1) * 6], in_=ptp6[:]
                            )
                        nc.sync.dma_start(
                            out=y_t[gi * 512 : (gi + 1) * 512, :].rearrange(
                                "(j p) f -> p j f", p=P
                            ),
                            in_=ytg[:].rearrange("p (j f) -> p j f", f=6),
                        )
                        continue
                    hc = rhpool.tile([HID, 512], bf16, tag="hc")
                    if r == 0:
                        nc.scalar.activation(
                            out=hc[:], in_=psum[:], func=AFT.Relu,
                            bias=b_in_sb[:],
                        )
                    else:
                        atc = atpool.tile([HID, 512], bf16, tag="at")
                        nc.scalar.activation(
                            out=atc[:], in_=psum[:], func=AFT.Copy,
                        )
                        pt = ps_tr.tile([HID, 512], f32, space="PSUM", tag="ptr")
                        nc.tensor.matmul(
                            out=pt[:],
                            lhsT=whid_sb[:, (r - 1) * HID : r * HID],
                            rhs=atc[:], start=True, stop=True,
                        )
                        nc.scalar.activation(
                            out=hc[:], in_=pt[:], func=AFT.Relu,
                            bias=bhid_sb[:, r - 1 : r],
                        )
                    htg = trpool.tile([P, 4 * FW], bf16, tag="ht")
                    nc.vector.memset(htg[:], 0.0)
                    for jj in range(4):
                        ptp = ps_tp.tile([P, HID], bf16, space="PSUM", tag="ptp")
                        nc.tensor.transpose(
                            out=ptp[:], in_=hc[:, jj * P : (jj + 1) * P],
                            identity=ident_b[0:HID, 0:HID],
                        )
                        nc.scalar.activation(
                            out=htg[:, jj * FW : jj * FW + HID], in_=ptp[:],
                            func=AFT.Copy,
                        )
                    q = G2Q[gi]
                    rb = gi * 512 - int(QB[q])
                    nc.sync.dma_start(
                        out=hq[tn][q][rb : rb + 512, :].rearrange(
                            "(j p) f -> p j f", p=P
                        ),
                        in_=htg[:].rearrange("p (j f) -> p j f", f=FW),
                    )
                    if gi in CGI:
                        cq = CGI[gi]
                        nc.gpsimd.collective_compute(
                            "AllGather", mybir.AluOpType.bypass,
                            replica_groups=rg,
                            ins=[hq[tn][cq][:]], outs=[tq[tn][cq][:, :]],
                        )
                    if gi == NG - 1:
                        nc.gpsimd.collective_compute(
                            "AllGather", mybir.AluOpType.bypass,
                            replica_groups=rg,
                            ins=[hq[tn][4][:]], outs=[tq[tn][4][:, :]],
                        )

    nc.compile()
    return nc


# ----------------------------------------------------------------- assembly --
def make_in_maps(inputs, pre):
    NHID = NROUNDS - 2
    x = np.asarray(inputs["x"], np.float32)
    xpad = np.zeros((NCORES * DPC, 3), np.float32)
    xpad[:N] = x
    w_in = np.asarray(inputs["W_in"], np.float32)
    b_in = np.asarray(inputs["b_in"], np.float32).reshape(HID, 1)
    w_hid = np.asarray(inputs["W_hid"], np.float32)[:NHID]
    b_hid = np.asarray(inputs["b_hid"], np.float32)[:NHID]
    w_out = np.asarray(inputs["W_out"], np.float32)
    b_out = np.asarray(inputs["b_out"], np.float32).reshape(6, 1)

    # idx wrapped-16 + replicated across the 8 Q7 cores
    idxw = []
    for k in range(NCORES):
        a = pre["idx"][k]               # [P, NCOL] slot layout (lane, col)
        flat = a.T.reshape(-1)          # pos order: pos = c*128 + p
        w16 = flat.reshape(-1, 16).T    # [16, NSLOT/16]
        idxw.append(np.ascontiguousarray(np.tile(w16, (8, 1))))

    in_maps = []
    for k in range(NCORES):
        in_maps.append(
            {
                "idx": idxw[k],
                "offt": np.ascontiguousarray(pre["off"][k]),
                "nrmt": np.ascontiguousarray(pre["nrm"][k]),
                "xsh": np.ascontiguousarray(xpad[k * DPC : (k + 1) * DPC]),
                "w_in": w_in,
                "b_in": b_in,
                "w_hid": w_hid.astype(ml_dtypes.bfloat16),
                "b_hid": np.ascontiguousarray(
                    b_hid.reshape(-1, HID, 1)
                ).astype(np.float32),
                "w_out": w_out.astype(ml_dtypes.bfloat16),
                "b_out": b_out,
            }
        )
    return in_maps


def run(inputs, **spmd_kwargs):
    edge_index = np.asarray(inputs["edge_index"])
    pre = preprocess(edge_index)
    nc = build_program(pre["sched"])
    in_maps = make_in_maps(inputs, pre)
    res = run_bass_kernel_spmd(
        nc, in_maps, core_ids=list(range(NCORES)), **spmd_kwargs
    )
    y = np.concatenate([res.results[k]["y"] for k in range(NCORES)])
    return y[:N].astype(np.float32), res


def kernel(**inputs):
    y, _ = run(inputs)
    return y


# revision 16
# speedup vs baseline: 1.1316x; 1.1316x over previous
"""GCNConvNet Trainium2 kernel (8 NeuronCores, Bass/Tile).

Dst-sharded graph parallelism, 8 aggregation rounds (A(HW) == (AH)W lets every
round aggregate 64-feature rows):
  - Node features live in an HBM table of bf16 rows padded to 256B (gather
    granule).  Each core owns 12800 destination rows.  The table is split into
    4 quarter tensors (per-core quarter slices interleaved) so the per-round
    AllGather is quartered and overlaps the next round's gathers.
  - Per round, each core gathers its edges' source rows with dma_gather
    (int16 indices; 4 SWDGE queues round-robined), then segment-sums them
    into 64-dst PSUM windows with TensorE matmuls against one-hot scatter
    blocks built ON-CHIP by the vector engine from compact (offset, norm)
    tables (symmetric-norm coefficients folded into the one-hot values).
  - Dense layer transform + bias/activation runs on each 512-dst chunk right
    after its aggregation; the updated chunk is transposed (TensorE) and
    published into the quarter slice; a quarter's AllGather fires as soon as
    its groups are published.
The block schedule is shared by all cores (single NEFF); per-core differences
live entirely in the input tensors (indices, off/norm tables, x shard).
"""

import sys

sys.path.insert(0, "/opt/trn_rl_repo")

import numpy as np
import ml_dtypes

import concourse.bacc as bacc
import concourse.mybir as mybir
import concourse.tile as tile
from concourse.bass_utils import run_bass_kernel_spmd
from concourse.masks import make_identity

P = 128
HID = 64
FW = 128          # table row width (bf16) = 256B gather granule; cols 64: pad
AFT = mybir.ActivationFunctionType
ALU = mybir.AluOpType

N = 100000
NCORES = 8
DPC = 12800       # dst rows per core
W = 64            # dsts per window
NW = 200          # windows per core
WPG = 8           # windows per psum group (512 cols)
NG = 25           # psum groups per core
NROUNDS = 8
J = 100           # 128-row tiles per core slice
NTOT = NCORES * DPC   # padded node count / table rows
BANK = 25600      # int16-reachable table rows per gather bank
NB = 4


# ---------------------------------------------------------------- host side --
def preprocess(edge_index):
    """Slot/scatter schedule shared by all cores + per-core idx / off / norm.

    Slot order: (psum-group g, bank b, window w, dst, edge).  Within each
    (g,b): per-window slot counts are equalized across cores (max), then the
    (g,b) range is padded to a multiple of 128.  Slot s maps to m-tile
    position (lane s%128, col s//128).

    Table row of node n is n itself; bank b covers rows [b*BANK, (b+1)*BANK).
    """
    src = np.concatenate([edge_index[0], np.arange(N)]).astype(np.int64)
    dst = np.concatenate([edge_index[1], np.arange(N)]).astype(np.int64)
    deg = np.bincount(dst, minlength=N).astype(np.float64)
    dinv = deg ** -0.5
    norm = (dinv[src] * dinv[dst]).astype(np.float32)

    core = dst // DPC
    r = dst % DPC
    win = r // W                    # global window id [0, NW)
    grp = win // WPG                # psum group [0, NG)
    dloc = r % W

    bank = src // BANK
    rowin = src % BANK              # row within bank

    # per-(core, g, b, w) counts -> equalized across cores
    key = ((core * NG + grp) * NB + bank) * NW + win
    cnt = np.bincount(key, minlength=NCORES * NG * NB * NW).reshape(
        NCORES, NG, NB, NW
    )
    cnt_eq = cnt.max(axis=0)                   # [NG, NB, NW]

    gb_sz = cnt_eq.sum(axis=2).reshape(-1)     # [NG*NB], ordered (g, b)
    gb_pad = (-gb_sz) % P
    gb_base = np.concatenate([[0], np.cumsum(gb_sz + gb_pad)])
    NSLOT = int(gb_base[-1])
    NCOL = NSLOT // P

    w_off = np.zeros_like(cnt_eq)
    w_off[:, :, 1:] = np.cumsum(cnt_eq, axis=2)[:, :, :-1]
    w_base = gb_base[:-1].reshape(NG, NB)[:, :, None] + w_off  # [NG,NB,NW]

    # edge -> slot
    order = np.lexsort((dst, win, bank, grp, core))
    srcs_o = src[order]
    rowin_o = rowin[order]
    norms_o = norm[order]
    cores_o, grps_o, banks_o, wins_o, dlocs_o = (
        core[order], grp[order], bank[order], win[order], dloc[order]
    )
    key_o = ((cores_o * NG + grps_o) * NB + banks_o) * NW + wins_o
    starts = np.zeros(NCORES * NG * NB * NW + 1, np.int64)
    np.add.at(starts, key_o + 1, 1)
    starts = np.cumsum(starts)
    rank = np.arange(len(key_o)) - starts[key_o]
    slot = w_base[grps_o, banks_o, wins_o] + rank

    lane = slot % P
    col = slot // P

    idx = np.zeros((NCORES, P, NCOL), np.int16)
    idx[cores_o, lane, col] = rowin_o.astype(np.int16)

    # ---- matmul schedule (shared) ----
    # entry per (g, b, w, col-touched); rhs is always S[:, m, 0:64].
    mm_col, mm_w, mm_g, mm_b = [], [], [], []
    for g in range(NG):
        for b in range(NB):
            for w in range(WPG * g, WPG * (g + 1)):
                base = int(w_base[g, b, w])
                n = int(cnt_eq[g, b, w])
                if n == 0:
                    continue
                for c in range(base // P, (base + n - 1) // P + 1):
                    mm_col.append(c)
                    mm_w.append(w)
                    mm_g.append(g)
                    mm_b.append(b)
    NMM = len(mm_col)
    first_of_w, last_of_w = {}, {}
    for m, w in enumerate(mm_w):
        if w not in first_of_w:
            first_of_w[w] = m
        last_of_w[w] = m
    mm_start = np.array([first_of_w[w] == m for m, w in enumerate(mm_w)])
    mm_stop = np.array([last_of_w[w] == m for m, w in enumerate(mm_w)])

    # compact S-build tables: off (dst offset in window) + norm per (lane, m)
    mm_of = {}
    for m in range(NMM):
        mm_of[(mm_col[m], mm_w[m])] = m
    pair_keys = col * NW + wins_o
    uniq, inv = np.unique(pair_keys, return_inverse=True)
    mm_u = np.array([mm_of[(int(pk) // NW, int(pk) % NW)] for pk in uniq])
    m_of_edge = mm_u[inv]
    off_arr = np.zeros((NCORES, P, NMM), np.float32)
    nrm_arr = np.zeros((NCORES, P, NMM), np.float32)
    off_arr[cores_o, lane, m_of_edge] = dlocs_o
    nrm_arr[cores_o, lane, m_of_edge] = norms_o

    groups = []
    for g in range(NG):
        gc0 = int(gb_base[g * NB] // P)
        gc1 = int(gb_base[(g + 1) * NB] // P) if g + 1 < NG else NCOL
        bank_cols = [
            (int(gb_base[g * NB + b] // P), int(gb_base[g * NB + b + 1] // P))
            for b in range(NB)
        ]
        mms = [m for m in range(NMM) if mm_g[m] == g]
        groups.append(
            dict(cols=(gc0, gc1), bank_cols=bank_cols,
                 mm0=min(mms), mm1=max(mms) + 1)
        )
    for g, gr in enumerate(groups):
        for m in range(gr["mm0"], gr["mm1"]):
            assert mm_g[m] == g

    sched = dict(
        mm_col=mm_col, mm_w=mm_w, mm_b=mm_b,
        mm_start=mm_start, mm_stop=mm_stop,
        groups=groups, NMM=NMM, NCOL=NCOL, NSLOT=NSLOT,
    )
    return dict(
        idx=idx,
        off=off_arr.astype(ml_dtypes.bfloat16),
        nrm=nrm_arr.astype(ml_dtypes.bfloat16),
        sched=sched,
    )


# -------------------------------------------------------------- device side --
def build_program(sched):
    f32, bf16, i16 = mybir.dt.float32, mybir.dt.bfloat16, mybir.dt.int16
    mm_col, mm_w, mm_b = sched["mm_col"], sched["mm_w"], sched["mm_b"]
    mm_start, mm_stop = sched["mm_start"], sched["mm_stop"]
    groups, NMM, NCOL = sched["groups"], sched["NMM"], sched["NCOL"]
    NHID = NROUNDS - 2

    nc = bacc.Bacc(
        "TRN2", target_bir_lowering=False, debug=False,
        num_devices=NCORES, num_swdge_queues=4,
    )

    idx_t = nc.dram_tensor("idx", [P, NCOL * 8], i16, kind="ExternalInput")
    off_t = nc.dram_tensor("offt", [P, NMM], bf16, kind="ExternalInput")
    nrm_t = nc.dram_tensor("nrmt", [P, NMM], bf16, kind="ExternalInput")
    x_t = nc.dram_tensor("xsh", [DPC, 3], f32, kind="ExternalInput")
    win_t = nc.dram_tensor("w_in", [3, HID], f32, kind="ExternalInput")
    bin_t = nc.dram_tensor("b_in", [HID, 1], f32, kind="ExternalInput")
    whid_t = nc.dram_tensor("w_hid", [NHID, HID, HID], bf16, kind="ExternalInput")
    bhid_t = nc.dram_tensor("b_hid", [NHID, HID, 1], f32, kind="ExternalInput")
    wout_t = nc.dram_tensor("w_out", [HID, 6], bf16, kind="ExternalInput")
    bout_t = nc.dram_tensor("b_out", [6, 1], f32, kind="ExternalInput")
    y_t = nc.dram_tensor("y", [DPC, 6], f32, kind="ExternalOutput")

    tables = [
        nc.dram_tensor(f"table{i}", [NTOT, FW], bf16, addr_space="Shared")
        for i in range(2)
    ]
    hsl = [nc.dram_tensor(f"hslice{i}", [DPC, FW], bf16) for i in range(2)]
    rg = [list(range(NCORES))]

    GC = max(gr["cols"][1] - gr["cols"][0] for gr in groups)
    SM = max(gr["mm1"] - gr["mm0"] for gr in groups)
    for gr in groups:
        for b in range(NB):
            c0, c1 = gr["bank_cols"][b]
            assert (c1 - c0) * P <= 8192

    with tile.TileContext(nc, num_cores=NCORES) as tc:
        with (
            tc.tile_pool(name="const", bufs=1) as cpool,
            tc.tile_pool(name="mp", bufs=2) as mpool,
            tc.tile_pool(name="sp", bufs=2) as spool,
            tc.tile_pool(name="ip", bufs=3) as ipool,
            tc.tile_pool(name="trp", bufs=2) as trpool,
            tc.tile_pool(name="t0p", bufs=1) as t0pool,
            tc.tile_pool(name="rhp", bufs=2) as rhpool,
            tc.tile_pool(name="atp", bufs=2) as atpool,
            tc.tile_pool(name="ps_sc", bufs=2, space="PSUM") as ps_sc,
            tc.tile_pool(name="ps_tr", bufs=2, space="PSUM") as ps_tr,
            tc.tile_pool(name="ps_tp", bufs=2, space="PSUM") as ps_tp,
        ):
            # ---- constants ----
            ident_f = cpool.tile([P, P], f32, tag="idf")
            make_identity(nc, ident_f[:])
            ident_b = cpool.tile([P, P], bf16, tag="idb")
            make_identity(nc, ident_b[:])
            w_in_sb = cpool.tile([3, HID], f32, tag="wi")
            nc.sync.dma_start(out=w_in_sb[:], in_=win_t[:])
            b_in_sb = cpool.tile([HID, 1], f32, tag="bi")
            nc.sync.dma_start(out=b_in_sb[:], in_=bin_t[:])
            whid_sb = cpool.tile([HID, NHID * HID], bf16, tag="wh")
            bhid_sb = cpool.tile([HID, NHID], f32, tag="bh")
            for l in range(NHID):
                nc.sync.dma_start(
                    out=whid_sb[:, l * HID : (l + 1) * HID], in_=whid_t[l, :, :]
                )
                nc.sync.dma_start(out=bhid_sb[:, l : l + 1], in_=bhid_t[l, :, :])
            wout_sb = cpool.tile([HID, 6], bf16, tag="wo")
            nc.sync.dma_start(out=wout_sb[:], in_=wout_t[:])
            bout_sb = cpool.tile([6, 1], f32, tag="bo")
            nc.sync.dma_start(out=bout_sb[:], in_=bout_t[:])
            iota_sb = cpool.tile([P, W], bf16, tag="io")
            nc.gpsimd.iota(
                iota_sb[:], pattern=[[1, W]], base=0, channel_multiplier=0,
                allow_small_or_imprecise_dtypes=True,
            )
            off_sb = cpool.tile([P, NMM], bf16, tag="of")
            nc.sync.dma_start(out=off_sb[:], in_=off_t[:])
            nrm_sb = cpool.tile([P, NMM], bf16, tag="nr")
            nc.sync.dma_start(out=nrm_sb[:], in_=nrm_t[:])

            # ---- round 0 table: t0 = x @ W_in ----
            t0 = t0pool.tile([P, J * FW], bf16, tag="t0")
            nc.vector.memset(t0[:], 0.0)
            for j in range(J):
                xc = rhpool.tile([P, 3], f32, tag="xc")
                nc.sync.dma_start(out=xc[:], in_=x_t[j * P : (j + 1) * P, :])
                pxT = ps_tp.tile([3, P], f32, space="PSUM", tag="ptp")
                nc.tensor.transpose(out=pxT[:], in_=xc[:], identity=ident_f[:])
                xT = rhpool.tile([3, P], f32, tag="xT")
                nc.vector.tensor_copy(out=xT[:], in_=pxT[:])
                pt0 = ps_tr.tile([P, HID], f32, space="PSUM", tag="ptr")
                nc.tensor.matmul(
                    out=pt0[:], lhsT=xT[:], rhs=w_in_sb[:], start=True,
                    stop=True,
                )
                nc.scalar.activation(
                    out=t0[:, j * FW : j * FW + HID], in_=pt0[:],
                    func=AFT.Copy,
                )
            nc.sync.dma_start(
                out=hsl[0].ap().rearrange("(j p) f -> p j f", p=P),
                in_=t0[:].rearrange("p (j f) -> p j f", f=FW),
            )
            nc.gpsimd.collective_compute(
                "AllGather", mybir.AluOpType.bypass, replica_groups=rg,
                ins=[hsl[0][:]], outs=[tables[0][:, :]],
            )

            # ---- rounds ----
            qn = 0
            for r in range(NROUNDS):
                ti, tn = r % 2, (r + 1) % 2
                for gi, gr in enumerate(groups):
                    gc0, gc1 = gr["cols"]
                    # gathers (4 banks -> one m tile)
                    idx_sb = ipool.tile([P, GC * 8], i16, tag="ix")
                    nc.sync.dma_start(
                        out=idx_sb[:, 0 : (gc1 - gc0) * 8],
                        in_=idx_t[:, gc0 * 8 : gc1 * 8],
                    )
                    mt = mpool.tile([P, GC, FW], bf16, tag="m")
                    for b in range(NB):
                        c0, c1 = gr["bank_cols"][b]
                        if c1 == c0:
                            continue
                        nidx = (c1 - c0) * P
                        nc.gpsimd.dma_gather(
                            out_ap=mt[:, c0 - gc0 : c1 - gc0, :],
                            in_ap=tables[ti][b * BANK : (b + 1) * BANK, :],
                            idxs_ap=idx_sb[:, (c0 - gc0) * 8 : (c1 - gc0) * 8],
                            num_idxs=nidx,
                            num_idxs_reg=nidx,
                            elem_size=FW,
                            single_packet=False,
                            queue_num=qn % 4,
                        )
                        qn += 1
                    # on-chip S build: S = (iota == off) * norm
                    mm0, mm1 = gr["mm0"], gr["mm1"]
                    nm = mm1 - mm0
                    sg = spool.tile([P, SM, W], bf16, tag="s")
                    iota_b = (
                        iota_sb[:]
                        .rearrange("p (o j) -> p o j", o=1)
                        .broadcast_to([P, nm, W])
                    )
                    off_b = (
                        off_sb[:, mm0:mm1]
                        .rearrange("p (m o) -> p m o", o=1)
                        .broadcast_to([P, nm, W])
                    )
                    nrm_b = (
                        nrm_sb[:, mm0:mm1]
                        .rearrange("p (m o) -> p m o", o=1)
                        .broadcast_to([P, nm, W])
                    )
                    nc.vector.scalar_tensor_tensor(
                        out=sg[:, 0:nm, :], in0=iota_b, scalar=0.0, in1=off_b,
                        op0=ALU.add, op1=ALU.is_equal,
                    )
                    nc.vector.scalar_tensor_tensor(
                        out=sg[:, 0:nm, :], in0=sg[:, 0:nm, :], scalar=1.0,
                        in1=nrm_b, op0=ALU.mult, op1=ALU.mult,
                    )
                    # aggregation matmuls
                    psum = ps_sc.tile([HID, 512], f32, space="PSUM", tag="psc")
                    for m in range(mm0, mm1):
                        c, w = mm_col[m], mm_w[m]
                        wl = w - WPG * gi
                        nc.tensor.matmul(
                            out=psum[:, wl * W : (wl + 1) * W],
                            lhsT=mt[:, c - gc0, 0:HID],
                            rhs=sg[:, m - mm0, 0:W],
                            start=bool(mm_start[m]),
                            stop=bool(mm_stop[m]),
                            skip_group_check=True,
                        )
                    # transform + transpose + publish for this 512-dst chunk
                    if r == NROUNDS - 1:
                        atc = atpool.tile([HID, 512], bf16, tag="at")
                        nc.scalar.activation(
                            out=atc[:], in_=psum[:], func=AFT.Copy,
                        )
                        yc = rhpool.tile([6, 512], f32, tag="yc")
                        pt6 = ps_tr.tile([6, 512], f32, space="PSUM", tag="ptr")
                        nc.tensor.matmul(
                            out=pt6[:], lhsT=wout_sb[:], rhs=atc[:],
                            start=True, stop=True,
                        )
                        nc.scalar.activation(
                            out=yc[:], in_=pt6[:], func=AFT.Sigmoid,
                            bias=bout_sb[:],
                        )
                        ytg = trpool.tile([P, 4 * 6], f32, tag="yt")
                        for jj in range(4):
                            ptp6 = ps_tp.tile([P, 6], f32, space="PSUM", tag="ptp")
                            nc.tensor.transpose(
                                out=ptp6[:], in_=yc[:, jj * P : (jj + 1) * P],
                                identity=ident_f[0:6, 0:6],
                            )
                            nc.vector.tensor_copy(
                                out=ytg[:, jj * 6 : (jj + 1) * 6], in_=ptp6[:]
                            )
                        nc.sync.dma_start(
                            out=y_t[gi * 512 : (gi + 1) * 512, :].rearrange(
                                "(j p) f -> p j f", p=P
                            ),
                            in_=ytg[:].rearrange("p (j f) -> p j f", f=6),
                        )
                        continue
                    hc = rhpool.tile([HID, 512], bf16, tag="hc")
                    if r == 0:
                        nc.scalar.activation(
                            out=hc[:], in_=psum[:], func=AFT.Relu,
                            bias=b_in_sb[:],
                        )
                    else:
                        atc = atpool.tile([HID, 512], bf16, tag="at")
                        nc.scalar.activation(
                            out=atc[:], in_=psum[:], func=AFT.Copy,
                        )
                        pt = ps_tr.tile([HID, 512], f32, space="PSUM", tag="ptr")
                        nc.tensor.matmul(
                            out=pt[:],
                            lhsT=whid_sb[:, (r - 1) * HID : r * HID],
                            rhs=atc[:], start=True, stop=True,
                        )
                        nc.scalar.activation(
                            out=hc[:], in_=pt[:], func=AFT.Relu,
                            bias=bhid_sb[:, r - 1 : r],
                        )
                    htg = trpool.tile([P, 4 * FW], bf16, tag="ht")
                    if r == 0 and gi < 2:
                        # pad cols (64:128) stay zero across buffer reuse
                        nc.vector.memset(htg[:], 0.0)
                    for jj in range(4):
                        ptp = ps_tp.tile([P, HID], bf16, space="PSUM", tag="ptp")
                        nc.tensor.transpose(
                            out=ptp[:], in_=hc[:, jj * P : (jj + 1) * P],
                            identity=ident_b[0:HID, 0:HID],
                        )
                        nc.scalar.activation(
                            out=htg[:, jj * FW : jj * FW + HID], in_=ptp[:],
                            func=AFT.Copy,
                        )
                    nc.sync.dma_start(
                        out=hsl[tn][gi * 512 : (gi + 1) * 512, :].rearrange(
                            "(j p) f -> p j f", p=P
                        ),
                        in_=htg[:].rearrange("p (j f) -> p j f", f=FW),
                    )
                    if gi == NG - 1:
                        nc.gpsimd.collective_compute(
                            "AllGather", mybir.AluOpType.bypass,
                            replica_groups=rg,
                            ins=[hsl[tn][:]], outs=[tables[tn][:, :]],
                        )

    nc.compile()
    return nc


# ----------------------------------------------------------------- assembly --
def make_in_maps(inputs, pre):
    NHID = NROUNDS - 2
    x = np.asarray(inputs["x"], np.float32)
    xpad = np.zeros((NCORES * DPC, 3), np.float32)
    xpad[:N] = x
    w_in = np.asarray(inputs["W_in"], np.float32)
    b_in = np.asarray(inputs["b_in"], np.float32).reshape(HID, 1)
    w_hid = np.asarray(inputs["W_hid"], np.float32)[:NHID]
    b_hid = np.asarray(inputs["b_hid"], np.float32)[:NHID]
    w_out = np.asarray(inputs["W_out"], np.float32)
    b_out = np.asarray(inputs["b_out"], np.float32).reshape(6, 1)

    # idx wrapped-16 + replicated across the 8 Q7 cores
    idxw = []
    for k in range(NCORES):
        a = pre["idx"][k]               # [P, NCOL] slot layout (lane, col)
        flat = a.T.reshape(-1)          # pos order: pos = c*128 + p
        w16 = flat.reshape(-1, 16).T    # [16, NSLOT/16]
        idxw.append(np.ascontiguousarray(np.tile(w16, (8, 1))))

    in_maps = []
    for k in range(NCORES):
        in_maps.append(
            {
                "idx": idxw[k],
                "offt": np.ascontiguousarray(pre["off"][k]),
                "nrmt": np.ascontiguousarray(pre["nrm"][k]),
                "xsh": np.ascontiguousarray(xpad[k * DPC : (k + 1) * DPC]),
                "w_in": w_in,
                "b_in": b_in,
                "w_hid": w_hid.astype(ml_dtypes.bfloat16),
                "b_hid": np.ascontiguousarray(
                    b_hid.reshape(-1, HID, 1)
                ).astype(np.float32),
                "w_out": w_out.astype(ml_dtypes.bfloat16),
                "b_out": b_out,
            }
        )
    return in_maps


def run(inputs, **spmd_kwargs):
    edge_index = np.asarray(inputs["edge_index"])
    pre = preprocess(edge_index)
    nc = build_program(pre["sched"])
    in_maps = make_in_maps(inputs, pre)
    res = run_bass_kernel_spmd(
        nc, in_maps, core_ids=list(range(NCORES)), **spmd_kwargs
    )
    y = np.concatenate([res.results[k]["y"] for k in range(NCORES)])
    return y[:N].astype(np.float32), res


def kernel(**inputs):
    y, _ = run(inputs)
    return y
